# revision 1
# baseline (speedup 1.0000x reference)
"""Block-causal attention (B=4, N=2048, C=1024, H=16, block=128) on 8 TRN2
NeuronCores — fp8 DoubleRow edition.

Sharding: core = 2*b + g (b in 0..3 batches, g in 0..1 head-groups of 8
heads). Dataflow mirrors the f32r baseline (feature-major q/k, token-major
v, out^T attention accumulation, per-core partial projection; host sums the
two half-feature partials), with the matmul work cut roughly in half:

- qkv and proj matmuls run in fp8e4 DoubleRow mode, contracting 256 rows
  per instruction (weights pre-scaled by 32 on host so fp8 quantization
  stays in the normal range; the 32*32 factor is divided out on host).
- attn@v runs as DoubleRow over key-tile PAIRS with a ones-augmented v
  (M=65): row 64 of each AV accumulator IS the softmax denominator, so the
  ones-matmul denominator pass of the baseline disappears entirely.
- The denominator row is copied to SBUF (DVE, f32r view), replicated
  across 64 partitions with a K=1 f32r matmul, reciprocal'd once per unit
  [64, 1024], and multiplied into the fp8 `at` tile that feeds proj.
- q/k tiles stay bf16 (same PE cost as f32r, keeps S logit noise low, and
  avoids f32r's 4 cycles/row penalty on N<256 moving dims).
- exp runs on ACT (exp(S)/4 to center fp8e4 range); an optional fraction
  is offloaded to DVE via a Schraudolph bitcast exp whose uint8 saturating
  conversion clamps the negative tail to +0.
"""

import numpy as np
import ml_dtypes
from collections import deque
from contextlib import ExitStack

B, N, C, H, HD = 4, 2048, 1024, 16, 64
HPC = 8               # heads per core
F = HPC * HD          # 512 features per core
NCORES = 8
SCALE = float(HD) ** -0.5
NT = N // 128         # 16 token tiles
NCH = 4               # token chunks of 512

QKV_FP8 = False
ET_FP8 = False
PROJ_FP8 = False
WSCALE = 32.0         # host pre-scale on wq/wk/wv/wp before fp8 quant
DVE_EXP_MOD = 4       # every DVE_EXP_MOD-th exp tile runs on DVE (0 = off)

LNK = 3.4657359027997265  # ln(32): exp(s)/32 keeps max logit 8.06 under fp8e4 max 240
LOG2E = 1.4426950408889634

_CACHE = {}


def _build():
    import concourse.mybir as mybir
    import concourse.tile as tile
    from concourse import bacc

    f32 = mybir.dt.float32
    f32r = mybir.dt.float32r
    bf16 = mybir.dt.bfloat16
    f8 = mybir.dt.float8e4
    u8 = mybir.dt.uint8
    i16 = mybir.dt.int16
    Exp = mybir.ActivationFunctionType.Exp
    DR = mybir.MatmulPerfMode.DoubleRow
    MULT = mybir.AluOpType.mult
    ADD = mybir.AluOpType.add

    qk_ws = WSCALE * WSCALE if QKV_FP8 else 1.0
    scale_eff = SCALE / qk_ws
    et_dt = f8 if ET_FP8 else bf16
    at_dt = f8 if PROJ_FP8 else bf16
    K8 = scale_eff * LOG2E * 8.0
    B8 = (7.0 - 5.0) * 8.0 - 0.5
    K16 = scale_eff * LOG2E * 128.0
    B16 = 127.0 * 128.0 - 0.0579 * 128.0 - 5.0 * 128.0  # incl. exp/32

    nc = bacc.Bacc("TRN2", target_bir_lowering=False, debug=False,
                   num_devices=NCORES)

    x_dt = f8 if QKV_FP8 else bf16
    xT = nc.dram_tensor("xT", [C, N], x_dt, kind="ExternalInput")
    wq = nc.dram_tensor("wq", [C, F], x_dt, kind="ExternalInput")
    wk = nc.dram_tensor("wk", [C, F], x_dt, kind="ExternalInput")
    wv = nc.dram_tensor("wv", [C, F], x_dt, kind="ExternalInput")
    wp = nc.dram_tensor("wp", [F, C], at_dt, kind="ExternalInput")
    out = nc.dram_tensor("out", [N, C], f32, kind="ExternalOutput")

    with tile.TileContext(nc) as tc, ExitStack() as ctx:
        persist = ctx.enter_context(tc.tile_pool(name="persist", bufs=1))
        xt_pool = ctx.enter_context(tc.tile_pool(name="xt", bufs=2))
        qt_pool = ctx.enter_context(tc.tile_pool(name="qt", bufs=2))
        et_pool = ctx.enter_context(tc.tile_pool(name="et", bufs=3))
        sm_pool = ctx.enter_context(tc.tile_pool(name="sums", bufs=2))
        rcf_pool = ctx.enter_context(tc.tile_pool(name="rcf", bufs=2))
        ost_pool = ctx.enter_context(tc.tile_pool(name="ost", bufs=2))
        ps_mm = ctx.enter_context(tc.tile_pool(name="ps_mm", bufs=2, space="PSUM"))
        ps_s = ctx.enter_context(tc.tile_pool(name="ps_s", bufs=2, space="PSUM"))
        ps_av = ctx.enter_context(tc.tile_pool(name="ps_av", bufs=1, space="PSUM"))

        # ---- persistent weights ----
        if QKV_FP8:
            wq_t = [persist.tile([128, 2, F], f8, name=f"wq{p}", tag=f"wq{p}")
                    for p in range(4)]
            wk_t = [persist.tile([128, 2, F], f8, name=f"wk{p}", tag=f"wk{p}")
                    for p in range(4)]
            wv_t = [persist.tile([128, 2, F], f8, name=f"wv{p}", tag=f"wv{p}")
                    for p in range(4)]
        else:
            wq_t = [persist.tile([128, F], bf16, name=f"wq{k}", tag=f"wq{k}")
                    for k in range(8)]
            wk_t = [persist.tile([128, F], bf16, name=f"wk{k}", tag=f"wk{k}")
                    for k in range(8)]
            wv_t = [persist.tile([128, F], bf16, name=f"wv{k}", tag=f"wv{k}")
                    for k in range(8)]
        if PROJ_FP8:
            wp_t = [persist.tile([128, 2, C], f8, name=f"wp{p}", tag=f"wp{p}")
                    for p in range(2)]
        else:
            wp_t = [persist.tile([128, C], bf16, name=f"wp{k}", tag=f"wp{k}")
                    for k in range(4)]
        ones1_t = persist.tile([1, 64], bf16, name="ones1", tag="ones1")
        bias_t = persist.tile([128, 1], f32, name="bias", tag="bias")

        # persistent k^T (bf16) and ones-augmented token-major v (pairs)
        kt_t = [[persist.tile([128, 512], bf16, name=f"kT{hp}_{jc}",
                              tag=f"kT{hp}_{jc}")
                 for jc in range(NCH)] for hp in range(4)]
        # 66-wide per-head v slices: 64 feature cols + ones col + pad col
        # (even head offsets/strides; dual-fp8 ldweights requires them)
        v_t = [persist.tile([128, 2, 8 * 66], et_dt, name=f"v{jp}",
                            tag=f"v{jp}") for jp in range(NT // 2)]
        at_t = [persist.tile([128, 4, 512], at_dt, name=f"at{c}", tag=f"at{c}")
                for c in range(NCH)]

        def load_weights():
            nc.vector.memset(ones1_t[:], 1.0)
            nc.vector.memset(bias_t[:], -LNK)
            for jp in range(NT // 2):
                # col 64 of each head's 65-wide v slice must be 1.0; the v
                # copies later only overwrite cols 0:64, so a full memset
                # up-front leaves the ones columns in place forever.
                nc.gpsimd.memset(v_t[jp][:], 1.0)
            if QKV_FP8:
                for p in range(4):
                    for i in range(2):
                        r0 = p * 256 + i * 128
                        nc.scalar.dma_start(wk_t[p][:, i, :], wk[r0:r0 + 128, :])
                        nc.gpsimd.dma_start(wv_t[p][:, i, :], wv[r0:r0 + 128, :])
                        nc.sync.dma_start(wq_t[p][:, i, :], wq[r0:r0 + 128, :])
            else:
                for k in range(8):
                    r0 = k * 128
                    nc.scalar.dma_start(wk_t[k][:], wk[r0:r0 + 128, :])
                    nc.gpsimd.dma_start(wv_t[k][:], wv[r0:r0 + 128, :])
                    nc.sync.dma_start(wq_t[k][:], wq[r0:r0 + 128, :])
            if PROJ_FP8:
                for p in range(2):
                    for i in range(2):
                        r0 = p * 256 + i * 128
                        nc.gpsimd.dma_start(wp_t[p][:, i, :], wp[r0:r0 + 128, :])
            else:
                for k in range(4):
                    nc.gpsimd.dma_start(wp_t[k][:], wp[k * 128:(k + 1) * 128, :])

        qt_state = {c: [] for c in range(NCH)}
        exp_ctr = [0]
        mm_ctr = [0]

        def mm_tile(c):
            # chunk 0 runs before any attention, so the ss banks are idle;
            # alternating into them deepens the qkv pipeline 2 -> 4 buffers.
            mm_ctr[0] += 1
            if c == 0 and mm_ctr[0] % 2 == 0:
                return ps_s.tile([128, 1024], f32, name="s", tag="s")[:, 0:512]
            return ps_mm.tile([128, 512], f32, name="mm", tag="mm")[:]

        def qkv_units(c):
            c0 = c * 512
            xt_c = []

            def load():
                if QKV_FP8:
                    for p in range(4):
                        xt = xt_pool.tile([128, 2, 512], f8, name=f"xt{p}",
                                          tag=f"xt{p}")
                        for i in range(2):
                            r0 = p * 256 + i * 128
                            nc.sync.dma_start(xt[:, i, :],
                                              xT[r0:r0 + 128, c0:c0 + 512])
                        xt_c.append(xt)
                else:
                    for k in range(8):
                        xt = xt_pool.tile([128, 512], bf16, name=f"xt{k}",
                                          tag=f"xt{k}")
                        nc.sync.dma_start(xt[:],
                                          xT[k * 128:(k + 1) * 128, c0:c0 + 512])
                        xt_c.append(xt)

            def qk_mm(ps, w_t, hp):
                if QKV_FP8:
                    for p in range(4):
                        nc.tensor.matmul(ps[:],
                                         w_t[p][:, :, hp * 128:(hp + 1) * 128],
                                         xt_c[p][:],
                                         start=(p == 0), stop=(p == 3),
                                         perf_mode=DR)
                else:
                    for k in range(8):
                        nc.tensor.matmul(ps[:],
                                         w_t[k][:, hp * 128:(hp + 1) * 128],
                                         xt_c[k][:],
                                         start=(k == 0), stop=(k == 7))

            def q_group(hp):
                def emit():
                    ps = mm_tile(c)
                    qk_mm(ps, wq_t, hp)
                    qt = qt_pool.tile([128, 512], bf16, name=f"qT{hp}",
                                      tag=f"qT{hp}")
                    nc.vector.tensor_copy(qt[:], ps[:])
                    qt_state[c].append(qt)
                return emit

            def k_group(hp):
                def emit():
                    ps = mm_tile(c)
                    qk_mm(ps, wk_t, hp)
                    nc.vector.tensor_copy(kt_t[hp][c][:], ps[:])
                return emit

            def v_group(tl):
                def emit():
                    t = 4 * c + tl
                    ps = mm_tile(c)
                    if QKV_FP8:
                        for p in range(4):
                            nc.tensor.matmul(ps[:],
                                             xt_c[p][:, :, tl * 128:(tl + 1) * 128],
                                             wv_t[p][:],
                                             start=(p == 0), stop=(p == 3),
                                             perf_mode=DR)
                    else:
                        for k in range(8):
                            nc.tensor.matmul(ps[:],
                                             xt_c[k][:, tl * 128:(tl + 1) * 128],
                                             wv_t[k][:],
                                             start=(k == 0), stop=(k == 7))
                    src = ps[:].rearrange("p (h e) -> p h e", e=64)
                    dst = v_t[t // 2][:, t % 2, :].rearrange(
                        "p (h e) -> p h e", e=66)[:, :, 0:64]
                    nc.vector.tensor_copy(dst, src)
                return emit

            units = [load]
            for hp in range(4):
                units.append(k_group(hp))
            for tl in range(4):
                units.append(v_group(tl))
            for hp in range(4):
                units.append(q_group(hp))
            return units

        def proj_units(c):
            units = []
            for tl in range(4):
                for n2 in range(2):
                    def emit(tl=tl, n2=n2):
                        t = 4 * c + tl
                        ps = ps_mm.tile([128, 512], f32, name="mm", tag="mm")
                        if PROJ_FP8:
                            for pr in range(2):
                                nc.tensor.matmul(
                                    ps[:],
                                    at_t[c][:, 2 * pr:2 * pr + 2,
                                            tl * 128:(tl + 1) * 128],
                                    wp_t[pr][:, :, n2 * 512:(n2 + 1) * 512],
                                    start=(pr == 0), stop=(pr == 1),
                                    perf_mode=DR)
                        else:
                            for k in range(4):
                                nc.tensor.matmul(
                                    ps[:],
                                    at_t[c][:, k, tl * 128:(tl + 1) * 128],
                                    wp_t[k][:, n2 * 512:(n2 + 1) * 512],
                                    start=(k == 0), stop=(k == 3))
                        ost = ost_pool.tile([128, 512], f32, name="ost",
                                            tag="ost")
                        nc.vector.tensor_copy(ost[:], ps[:])
                        nc.sync.dma_start(
                            out[t * 128:(t + 1) * 128,
                                n2 * 512:(n2 + 1) * 512],
                            ost[:])
                    units.append(emit)
            return units

        units0 = qkv_units(0)
        units0[0]()
        load_weights()
        for u in units0[1:]:
            u()

        def emit_exp(ss, et, i, vco):
            src = ss[:].rearrange("p (h q) -> p h q", h=2)[:, :, vco:512]
            dst = et[:, i, :].rearrange("p (h q) -> p h q", h=2)[:, :, vco:512]
            exp_ctr[0] += 1
            on_dve = DVE_EXP_MOD and (exp_ctr[0] % DVE_EXP_MOD == 0)
            if on_dve:
                if ET_FP8:
                    nc.vector.tensor_scalar(dst.bitcast(u8), src, K8, B8,
                                            MULT, ADD)
                else:
                    nc.vector.tensor_scalar(dst.bitcast(i16), src, K16, B16,
                                            MULT, ADD)
            else:
                nc.scalar.activation(dst, src, Exp, bias=bias_t[:],
                                     scale=scale_eff)

        def attn_unit(c, hp, fillers, stride=2):
            njp = 2 * c + 2
            qt_c = qt_state[c]
            av = [ps_av.tile([128, 512], f32, name=f"av{h}", tag=f"av{h}")
                  for h in range(2)]
            for jp in range(njp):
                first, last = (jp == 0), (jp == njp - 1)
                et = et_pool.tile([128, 2, 1024], et_dt, name="e", tag="e")
                vcos = []
                for i in range(2):
                    j = 2 * jp + i
                    jd = j - 4 * c
                    vco = jd * 128 if jd > 0 else 0
                    vcos.append(vco)
                    kt = kt_t[hp][j // 4]
                    kc = (j % 4) * 128
                    ss = ps_s.tile([128, 1024], f32, name="s", tag="s")
                    nc.tensor.matmul(ss[:, vco:512],
                                     kt[0:64, kc:kc + 128],
                                     qt_c[hp][0:64, vco:512],
                                     start=True, stop=True)
                    nc.tensor.matmul(ss[:, 512 + vco:1024],
                                     kt[64:128, kc:kc + 128],
                                     qt_c[hp][64:128, vco:512],
                                     start=True, stop=True)
                    emit_exp(ss, et, i, vco)
                qlo = vcos[1]
                for h in range(2):
                    gh = 2 * hp + h
                    co = h * 512
                    if ET_FP8:
                        if vcos[1] > vcos[0]:
                            v0 = v_t[jp][:, 0, gh * 66:gh * 66 + 65]
                            e0 = et[:, 0, co + vcos[0]:co + vcos[1]]
                            nc.tensor.matmul(av[h][0:65, vcos[0]:vcos[1]],
                                             v0, e0, start=first, stop=False)
                        vsl = v_t[jp][:, :, gh * 66:gh * 66 + 65]
                        esl = et[:, :, co + qlo:co + 512]
                        nc.tensor.matmul(av[h][0:65, qlo:512], vsl, esl,
                                         start=first, stop=last, perf_mode=DR)
                    else:
                        for i in range(2):
                            vco = vcos[i]
                            vsl = v_t[jp][:, i, gh * 66:gh * 66 + 65]
                            esl = et[:, i, co + vco:co + 512]
                            nc.tensor.matmul(av[h][0:65, vco:512], vsl, esl,
                                             start=(first and i == 0),
                                             stop=(last and i == 1))
                if jp % stride == stride - 1 and fillers:
                    fillers.popleft()()
            # keep the PE fed while the DVE sums-copy / recip chain runs:
            # pop fillers BEFORE the rc_rep matmuls (PE executes in emission
            # order, so anything emitted after rc_rep would stall behind it)
            if fillers:
                fillers.popleft()()
            # ---- normalization: row 64 of av[h] is the denominator ----
            rr = ps_s.tile([128, 1024], f32, name="s", tag="s")
            for h in range(2):
                sums = sm_pool.tile([1, 512], bf16, name=f"sum{h}",
                                    tag=f"sum{h}")
                nc.vector.tensor_copy(sums[:], av[h][64:65, :])
                nc.tensor.matmul(rr[0:64, h * 512:h * 512 + 512],
                                 ones1_t[:], sums[:],
                                 start=True, stop=True)
            rcf = rcf_pool.tile([64, 1024], f32, name="rcf", tag="rcf")
            nc.vector.reciprocal_approx_fast(rcf[:], rr[0:64, :])
            for h in range(2):
                nc.vector.tensor_mul(at_t[c][h * 64:(h + 1) * 64, hp, :],
                                     av[h][0:64, :],
                                     rcf[:, h * 512:(h + 1) * 512])

        phases = [
            ([(0, 0), (0, 1), (0, 2), (0, 3)], qkv_units(1)),
            ([(1, 0), (1, 1), (1, 2), (1, 3)], qkv_units(2)),
            ([(2, 0), (2, 1), (2, 2), (2, 3)], qkv_units(3)),
            ([(3, 0), (3, 1), (3, 2), (3, 3)],
             proj_units(0) + proj_units(1) + proj_units(2)),
        ]
        for units, filler_list in phases:
            fillers = deque(filler_list)
            total_jp = sum(2 * c + 2 for c, hp in units)
            stride = max(1, -(-total_jp // max(1, len(filler_list))))
            for (c, hp) in units:
                attn_unit(c, hp, fillers, stride)
            while fillers:
                fillers.popleft()()

        for u in proj_units(NCH - 1):
            u()

    nc.compile()
    return nc


def _get_nc():
    if "nc" not in _CACHE:
        _CACHE["nc"] = _build()
    return _CACHE["nc"]


def _in_maps(x, w_qkv, w_proj):
    f8np = ml_dtypes.float8_e4m3
    bf16np = ml_dtypes.bfloat16
    x_np = f8np if QKV_FP8 else bf16np
    at_np = f8np if PROJ_FP8 else bf16np
    wsc = WSCALE if QKV_FP8 else 1.0
    wsp = WSCALE if PROJ_FP8 else 1.0

    wr = w_qkv.reshape(C, 3, H, HD)
    wpr = w_proj.reshape(H, HD, C)
    maps = []
    for core in range(NCORES):
        b, g = core // 2, core % 2
        hs = slice(g * HPC, (g + 1) * HPC)
        maps.append({
            "xT": np.ascontiguousarray(x[b].T).astype(x_np),
            "wq": (wr[:, 0, hs, :].reshape(C, F) * wsc).astype(x_np),
            "wk": (wr[:, 1, hs, :].reshape(C, F) * wsc).astype(x_np),
            "wv": (wr[:, 2, hs, :].reshape(C, F) * wsc).astype(x_np),
            "wp": (wpr[hs].reshape(F, C) * wsp).astype(at_np),
        })
    return maps


def kernel(x, w_qkv, w_proj, b_proj, _trace=False):
    from concourse.bass_utils import run_bass_kernel_spmd

    x = np.asarray(x, dtype=np.float32)
    w_qkv = np.asarray(w_qkv, dtype=np.float32)
    w_proj = np.asarray(w_proj, dtype=np.float32)
    b_proj = np.asarray(b_proj, dtype=np.float32)

    nc = _get_nc()
    in_maps = _in_maps(x, w_qkv, w_proj)
    try:
        res = run_bass_kernel_spmd(nc, in_maps, list(range(NCORES)),
                                   trace=_trace)
    except Exception:
        try:
            import ctypes
            import jax
            lib = ctypes.CDLL("/opt/axon/libaxon_pjrt.so")
            jax.devices()
            lib.axon_reset.restype = ctypes.c_int64
            lib.axon_reset()
        except Exception:
            pass
        res = run_bass_kernel_spmd(nc, in_maps, list(range(NCORES)),
                                   trace=_trace)
    unscale = 1.0
    if QKV_FP8:
        unscale *= WSCALE        # v path carries one WSCALE into at
    if PROJ_FP8:
        unscale *= WSCALE        # wp carries another
    out = np.empty((B, N, C), dtype=np.float32)
    for b in range(B):
        out[b] = res.results[2 * b]["out"] + res.results[2 * b + 1]["out"]
    if unscale != 1.0:
        out *= 1.0 / unscale
    out += b_proj.reshape(1, 1, C)
    if _trace:
        return out, res
    return out

